# revision 4
# baseline (speedup 1.0000x reference)
"""BoundaryLoss Trainium2 kernel.

Computes mean((B(softmax(pred)) - B(onehot(target)))^2) where B is
clip(|3x3-Laplacian|, 0, 1) per (batch, class) plane.

Strategy (data parallel over batch, one batch element per NeuronCore):
  - rows-on-partitions layout; H=512 split into 5 bands (126+126+126+126+8
    output rows), each band loads its input rows plus halo.
  - softmax over classes: ACT Exp (f32->bf16), DVE tensor_reduce over c,
    DVE reciprocal_approx_fast, DVE broadcast multiply.
  - Laplacian = 9x - S_h(S_w(x)) via 3 TensorEngine matmuls per plane with
    banded weight matrices (S_h on partitions via the band matrix, S_w via
    rhs free-dim offsets +-1), accumulated in PSUM.
  - onehot planes generated on GPSIMD (is_equal vs class id).
  - pb/tb: ACT Abs evacuates PSUM; DVE min/sub/accumulate produce the
    per-partition sum of (pb - tb)^2; final mean on host.
"""

import os
import numpy as np
import ml_dtypes
from contextlib import ExitStack

import concourse.bass as bass
import concourse.tile as tile
from concourse import bacc, mybir
from concourse.bass_utils import run_bass_kernel_spmd

N_CORES = int(os.environ.get("K_CORES", "8"))
STAGE = int(os.environ.get("K_STAGE", "99"))
B, C, H, W = 8, 19, 512, 512
dt = mybir.dt

# band = (h_in_lo, P_in, M_out, shift)
BANDS = [
    (0, 128, 126, 0),
    (125, 128, 126, 1),
    (251, 128, 126, 1),
    (377, 128, 126, 1),
    (503, 9, 8, 1),
]


def _band_weights(P_in, M_out, shift):
    A = np.zeros((P_in, M_out), dtype=np.float32)
    E = np.zeros((P_in, M_out), dtype=np.float32)
    for m in range(M_out):
        for k in range(P_in):
            if abs(k - (m + shift)) <= 1:
                A[k, m] = 1.0
        E[m + shift, m] = 1.0
    w0 = (9.0 * E - A).astype(ml_dtypes.bfloat16)
    w1 = (-A).astype(ml_dtypes.bfloat16)
    return w0, w1


_NC_CACHE = None


def _build():
    global _NC_CACHE
    if _NC_CACHE is not None:
        return _NC_CACHE

    nc = bacc.Bacc("TRN2", target_bir_lowering=False, debug=False,
                   num_devices=N_CORES)

    pred_ap = nc.dram_tensor("pred", [C, H, W], dt.float32,
                             kind="ExternalInput").ap()
    tgt_ap = nc.dram_tensor("target", [H, W], dt.int32,
                            kind="ExternalInput").ap()
    out_ap = nc.dram_tensor("out", [128, 1], dt.float32,
                            kind="ExternalOutput").ap()

    # unique weight matrices: band0 (shift 0), mid (shift 1), last (9x8)
    w_drams = {}
    for key, (P_in, M_out, shift) in {
        "first": (128, 126, 0),
        "mid": (128, 126, 1),
        "last": (9, 8, 1),
    }.items():
        w0, w1 = _band_weights(P_in, M_out, shift)
        w_drams[key] = (nc.inline_tensor(w0, name=f"w0_{key}"),
                        nc.inline_tensor(w1, name=f"w1_{key}"))

    pred_v = pred_ap.transpose([1, 0, 2])  # [H, C, W] view of DRAM

    with tile.TileContext(nc) as tc:
        with ExitStack() as ctx:
            pool_pred = ctx.enter_context(tc.tile_pool(name="pred", bufs=2))
            pool_tgt = ctx.enter_context(tc.tile_pool(name="tgt", bufs=2))
            pool_big = ctx.enter_context(tc.tile_pool(name="big", bufs=1))
            pool_sm = ctx.enter_context(tc.tile_pool(name="sm", bufs=2))
            pool_cst = ctx.enter_context(tc.tile_pool(name="cst", bufs=1))
            pool_ps = ctx.enter_context(
                tc.tile_pool(name="ps", bufs=2, space="PSUM"))

            # weights to SBUF once
            w_sb = {}
            for key, (w0d, w1d) in w_drams.items():
                kk, mm = w0d.shape
                w0t = pool_cst.tile([kk, mm], dt.bfloat16, tag=f"w0{key}")
                w1t = pool_cst.tile([kk, mm], dt.bfloat16, tag=f"w1{key}")
                nc.sync.dma_start(w0t[:], w0d.ap()[:])
                nc.sync.dma_start(w1t[:], w1d.ap()[:])
                w_sb[key] = (w0t, w1t)

            acc = pool_cst.tile([128, len(BANDS)], dt.float32, tag="acc")
            nc.vector.memset(acc[:], 0.0)

            for bi, (h_lo, P_in, M_out, shift) in enumerate(BANDS):
                key = "first" if bi == 0 else ("last" if P_in < 128 else "mid")
                w0t, w1t = w_sb[key]

                predt = pool_pred.tile([128, C, W], dt.float32, tag="pred")
                nc.sync.dma_start(predt[0:P_in], pred_v[h_lo:h_lo + P_in])
                tgtt = pool_tgt.tile([128, W], dt.int32, tag="tgt")
                nc.sync.dma_start(tgtt[0:P_in], tgt_ap[h_lo:h_lo + P_in])

                # softmax over c
                if STAGE < 2:
                    continue
                e = pool_big.tile([128, C, W], dt.bfloat16, tag="e")
                nc.scalar.activation(e[0:P_in], predt[0:P_in],
                                     mybir.ActivationFunctionType.Exp)
                S = pool_sm.tile([128, W], dt.float32, tag="S")
                nc.vector.tensor_reduce(
                    S[0:P_in], e[0:P_in].transpose([0, 2, 1]),
                    axis=mybir.AxisListType.X, op=mybir.AluOpType.add)
                Rf = pool_sm.tile([128, W], dt.float32, tag="Rf")
                nc.vector.reciprocal_approx_fast(out=Rf[0:P_in], in_=S[0:P_in])
                Rb = pool_sm.tile([128, W], dt.bfloat16, tag="Rb")
                nc.vector.tensor_copy(Rb[0:P_in], Rf[0:P_in])
                p = pool_big.tile([128, C, W], dt.bfloat16, tag="p")
                nc.vector.tensor_tensor(
                    out=p[0:P_in], in0=e[0:P_in],
                    in1=Rb[0:P_in].unsqueeze(1).broadcast_to([P_in, C, W]),
                    op=mybir.AluOpType.mult)

                # onehot on gpsimd
                if STAGE < 3:
                    continue
                oh = pool_big.tile([128, C, W], dt.bfloat16, tag="oh")
                for c in range(C):
                    nc.gpsimd.tensor_scalar(
                        out=oh[0:P_in, c, :], in0=tgtt[0:P_in], scalar1=c,
                        scalar2=None, op0=mybir.AluOpType.is_equal)

                if STAGE < 4:
                    continue
                qp = pool_big.tile([128, C, W], dt.bfloat16, tag="qp")
                qt = pool_big.tile([128, C, W], dt.bfloat16, tag="qt")

                for c in range(C):
                    psp = pool_ps.tile([126, W], dt.float32, tag="pp")
                    pst = pool_ps.tile([126, W], dt.float32, tag="pt")
                    rp = p[0:P_in, c, :]
                    rt = oh[0:P_in, c, :]
                    nc.tensor.matmul(psp[0:M_out], lhsT=w0t[:], rhs=rp,
                                     start=True, stop=False)
                    nc.tensor.matmul(pst[0:M_out], lhsT=w0t[:], rhs=rt,
                                     start=True, stop=False)
                    nc.tensor.matmul(psp[0:M_out, 1:W], lhsT=w1t[:],
                                     rhs=p[0:P_in, c, 0:W - 1],
                                     start=False, stop=False)
                    nc.tensor.matmul(psp[0:M_out, 0:W - 1], lhsT=w1t[:],
                                     rhs=p[0:P_in, c, 1:W],
                                     start=False, stop=True)
                    nc.tensor.matmul(pst[0:M_out, 1:W], lhsT=w1t[:],
                                     rhs=oh[0:P_in, c, 0:W - 1],
                                     start=False, stop=False)
                    nc.tensor.matmul(pst[0:M_out, 0:W - 1], lhsT=w1t[:],
                                     rhs=oh[0:P_in, c, 1:W],
                                     start=False, stop=True)
                    nc.scalar.activation(qp[0:M_out, c, :], psp[0:M_out],
                                         mybir.ActivationFunctionType.Abs)
                    nc.scalar.activation(qt[0:M_out, c, :], pst[0:M_out],
                                         mybir.ActivationFunctionType.Abs)

                # tb = min(qt, 1); d = min(qp, 1) - tb; acc += sum(d^2)
                if STAGE < 5:
                    continue
                tb = pool_big.tile([128, C, W], dt.bfloat16, tag="e")
                nc.vector.tensor_scalar(
                    out=tb[0:M_out], in0=qt[0:M_out], scalar1=1.0,
                    scalar2=None, op0=mybir.AluOpType.min)
                if STAGE < 6:
                    continue
                d = pool_big.tile([128, C, W], dt.bfloat16, tag="oh")
                nc.vector.scalar_tensor_tensor(
                    out=d[0:M_out], in0=qp[0:M_out], scalar=1.0,
                    in1=tb[0:M_out], op0=mybir.AluOpType.min,
                    op1=mybir.AluOpType.subtract)
                if STAGE < 7:
                    continue
                sq = pool_big.tile([128, C, W], dt.bfloat16, tag="p")
                nc.vector.scalar_tensor_tensor(
                    out=sq[0:M_out], in0=d[0:M_out], scalar=1.0,
                    in1=d[0:M_out], op0=mybir.AluOpType.mult,
                    op1=mybir.AluOpType.mult,
                    accum_out=acc[0:M_out, bi:bi + 1])

            tot = pool_cst.tile([128, 1], dt.float32, tag="tot")
            nc.vector.tensor_reduce(tot[:], acc[:],
                                    axis=mybir.AxisListType.X,
                                    op=mybir.AluOpType.add)
            nc.sync.dma_start(out_ap[:], tot[:])

    nc.compile()
    _NC_CACHE = nc
    return nc


def kernel(pred: np.ndarray, target: np.ndarray) -> np.ndarray:
    assert pred.shape == (B, C, H, W) and target.shape == (B, H, W)
    nc = _build()
    in_maps = [
        {"pred": np.ascontiguousarray(pred[b]),
         "target": np.ascontiguousarray(target[b])}
        for b in range(N_CORES)
    ]
    res = run_bass_kernel_spmd(nc, in_maps, list(range(N_CORES)))
    total = sum(float(r["out"].sum()) for r in res.results)
    return np.float32(total / (B * C * H * W))


# revision 5
# speedup vs baseline: 2.9949x; 2.9949x over previous
"""BoundaryLoss Trainium2 kernel.

Computes mean((B(softmax(pred)) - B(onehot(target)))^2) where B is
clip(|3x3-Laplacian|, 0, 1) per (batch, class) plane.

Data parallel over batch: one batch element per NeuronCore (8 cores).
Per core, rows-on-partitions layout; H=512 in 5 bands (126*4+8 output rows),
each band loads its input rows plus halo. The Laplacian 9x - S_h(S_w(x)) is
3 TensorE matmuls per plane: banded weights give S_h over partitions, rhs
free-dim offsets +-1 give S_w. ACT evacuates |y| from PSUM two planes at a
time; DVE does softmax (bf16 tree sum, Ln/Exp reciprocal on ACT), onehot
(tensor_scalar is_equal on bf16 labels), clip, diff and the d^2 accumulation.
Host sums the per-partition partials and divides.
"""

import os
import numpy as np
import ml_dtypes
from contextlib import ExitStack

import concourse.bass as bass
import concourse.tile as tile
from concourse import bacc, mybir
from concourse.bass_utils import run_bass_kernel_spmd

N_CORES = int(os.environ.get("K_CORES", "8"))
B, C, H, W = 8, 19, 512, 512
dt = mybir.dt
AF = mybir.ActivationFunctionType
OP = mybir.AluOpType

# band = (h_in_lo, P_in, M_out, shift)
BANDS = [
    (0, 128, 126, 0),
    (125, 128, 126, 1),
    (251, 128, 126, 1),
    (377, 128, 126, 1),
    (503, 9, 8, 1),
]

# class pairs for 2-plane PSUM groups
PAIRS = [(c, c + 1) for c in range(0, C - 1, 2)] + [(C - 1,)]


def _band_weights(P_in, M_out, shift):
    A = np.zeros((P_in, M_out), dtype=np.float32)
    E = np.zeros((P_in, M_out), dtype=np.float32)
    for m in range(M_out):
        for k in range(P_in):
            if abs(k - (m + shift)) <= 1:
                A[k, m] = 1.0
        E[m + shift, m] = 1.0
    w0 = (9.0 * E - A).astype(ml_dtypes.bfloat16)
    w1 = (-A).astype(ml_dtypes.bfloat16)
    return w0, w1


_NC_CACHE = None


def _build():
    global _NC_CACHE
    if _NC_CACHE is not None:
        return _NC_CACHE

    nc = bacc.Bacc("TRN2", target_bir_lowering=False, debug=False,
                   num_devices=N_CORES)

    pred_ap = nc.dram_tensor("pred", [C, H, W], dt.float32,
                             kind="ExternalInput").ap()
    tgt_ap = nc.dram_tensor("target", [H, W], dt.int32,
                            kind="ExternalInput").ap()
    out_ap = nc.dram_tensor("out", [128, 1], dt.float32,
                            kind="ExternalOutput").ap()

    w_drams = {}
    for key, (P_in, M_out, shift) in {
        "first": (128, 126, 0),
        "mid": (128, 126, 1),
        "last": (9, 8, 1),
    }.items():
        w0, w1 = _band_weights(P_in, M_out, shift)
        w_drams[key] = (nc.inline_tensor(w0, name=f"w0_{key}"),
                        nc.inline_tensor(w1, name=f"w1_{key}"))

    pred_v = pred_ap.transpose([1, 0, 2])  # [H, C, W] view of DRAM

    with tile.TileContext(nc) as tc:
        with ExitStack() as ctx:
            pool_pred = ctx.enter_context(tc.tile_pool(name="pred", bufs=2))
            pool_tgt = ctx.enter_context(tc.tile_pool(name="tgt", bufs=2))
            pool_big = ctx.enter_context(tc.tile_pool(name="big", bufs=1))
            pool_sm = ctx.enter_context(tc.tile_pool(name="sm", bufs=2))
            pool_cst = ctx.enter_context(tc.tile_pool(name="cst", bufs=1))
            pool_ps = ctx.enter_context(
                tc.tile_pool(name="ps", bufs=2, space="PSUM"))

            w_sb = {}
            for key, (w0d, w1d) in w_drams.items():
                kk, mm = w0d.shape
                w0t = pool_cst.tile([kk, mm], dt.bfloat16, tag=f"w0{key}")
                w1t = pool_cst.tile([kk, mm], dt.bfloat16, tag=f"w1{key}")
                nc.sync.dma_start(w0t[:], w0d.ap()[:])
                nc.sync.dma_start(w1t[:], w1d.ap()[:])
                w_sb[key] = (w0t, w1t)

            acc = pool_cst.tile([128, len(BANDS)], dt.float32, tag="acc")
            nc.vector.memset(acc[:], 0.0)

            for bi, (h_lo, P_in, M_out, shift) in enumerate(BANDS):
                key = "first" if bi == 0 else ("last" if P_in < 128 else "mid")
                w0t, w1t = w_sb[key]
                Pi, Mo = P_in, M_out

                predt = pool_pred.tile([128, C, W], dt.float32, tag="pred")
                nc.sync.dma_start(predt[0:Pi], pred_v[h_lo:h_lo + Pi])
                tgtt = pool_tgt.tile([128, W], dt.int32, tag="tgt")
                nc.sync.dma_start(tgtt[0:Pi], tgt_ap[h_lo:h_lo + Pi])

                # ---- softmax over c ----
                e = pool_big.tile([128, C, W], dt.bfloat16, tag="e")
                nc.scalar.activation(e[0:Pi], predt[0:Pi], AF.Exp)

                # bf16 pairwise-tree sum of the 19 planes into scratch (qp tag)
                ts = pool_big.tile([128, C, W], dt.bfloat16, tag="qp")
                nc.vector.tensor_tensor(out=ts[0:Pi, 0:9, :], in0=e[0:Pi, 0:9, :],
                                        in1=e[0:Pi, 9:18, :], op=OP.add)
                nc.vector.tensor_tensor(out=ts[0:Pi, 0:4, :], in0=ts[0:Pi, 0:4, :],
                                        in1=ts[0:Pi, 4:8, :], op=OP.add)
                nc.vector.tensor_tensor(out=ts[0:Pi, 0:2, :], in0=ts[0:Pi, 0:2, :],
                                        in1=ts[0:Pi, 2:4, :], op=OP.add)
                nc.vector.tensor_tensor(out=ts[0:Pi, 0, :], in0=ts[0:Pi, 0, :],
                                        in1=ts[0:Pi, 1, :], op=OP.add)
                nc.vector.tensor_tensor(out=ts[0:Pi, 0, :], in0=ts[0:Pi, 0, :],
                                        in1=ts[0:Pi, 8, :], op=OP.add)
                S = pool_sm.tile([128, W], dt.float32, tag="S")
                nc.vector.tensor_tensor(out=S[0:Pi], in0=ts[0:Pi, 0, :],
                                        in1=e[0:Pi, 18, :], op=OP.add)

                # R = 1/S via exp(-ln(S)) on ACT, straight to bf16
                lnS = pool_sm.tile([128, W], dt.float32, tag="lnS")
                nc.scalar.activation(lnS[0:Pi], S[0:Pi], AF.Ln)
                Rb = pool_sm.tile([128, W], dt.bfloat16, tag="Rb")
                nc.scalar.activation(Rb[0:Pi], lnS[0:Pi], AF.Exp, scale=-1.0)

                # p = e * R, per class (keeps DVE in 2x bf16 mode)
                p = pool_big.tile([128, C, W], dt.bfloat16, tag="p")
                for c in range(C):
                    nc.vector.tensor_tensor(out=p[0:Pi, c, :], in0=e[0:Pi, c, :],
                                            in1=Rb[0:Pi], op=OP.mult)

                # ---- onehot: bf16 labels, TS is_equal per class (4x mode) ----
                tgtb = pool_sm.tile([128, W], dt.bfloat16, tag="tgtb")
                nc.vector.tensor_copy(tgtb[0:Pi], tgtt[0:Pi])
                oh = pool_big.tile([128, C, W], dt.bfloat16, tag="oh")
                for c in range(C):
                    nc.vector.tensor_scalar(out=oh[0:Pi, c, :], in0=tgtb[0:Pi],
                                            scalar1=float(c), scalar2=None,
                                            op0=OP.is_equal)

                # ---- Laplacian on PE, two planes per PSUM group ----
                qp = pool_big.tile([128, C, W], dt.bfloat16, tag="qp")
                qt = pool_big.tile([128, C, W], dt.bfloat16, tag="qt")
                for pr in PAIRS:
                    pp = pool_ps.tile([126, 2, W], dt.float32, tag="pp")
                    pt = pool_ps.tile([126, 2, W], dt.float32, tag="pt")
                    for j, c in enumerate(pr):
                        nc.tensor.matmul(pp[0:Mo, j, :], lhsT=w0t[:],
                                         rhs=p[0:Pi, c, :],
                                         start=True, stop=False)
                        nc.tensor.matmul(pt[0:Mo, j, :], lhsT=w0t[:],
                                         rhs=oh[0:Pi, c, :],
                                         start=True, stop=False)
                    for src, ps in ((p, pp), (oh, pt)):
                        for j, c in enumerate(pr):
                            last = j == len(pr) - 1
                            nc.tensor.matmul(ps[0:Mo, j, 1:W], lhsT=w1t[:],
                                             rhs=src[0:Pi, c, 0:W - 1],
                                             start=False, stop=False)
                            nc.tensor.matmul(ps[0:Mo, j, 0:W - 1], lhsT=w1t[:],
                                             rhs=src[0:Pi, c, 1:W],
                                             start=False, stop=last)
                    n = len(pr)
                    c0 = pr[0]
                    nc.scalar.activation(qp[0:Mo, c0:c0 + n, :],
                                         pp[0:Mo, 0:n, :], AF.Abs)
                    nc.scalar.activation(qt[0:Mo, c0:c0 + n, :],
                                         pt[0:Mo, 0:n, :], AF.Abs)

                # ---- tb = min(qt,1); pb = min(qp,1); acc += sum((pb-tb)^2) ----
                tb = pool_big.tile([128, C, W], dt.bfloat16, tag="e")
                nc.vector.tensor_scalar(out=tb[0:Mo], in0=qt[0:Mo],
                                        scalar1=1.0, scalar2=None, op0=OP.min)
                pb = pool_big.tile([128, C, W], dt.bfloat16, tag="p")
                nc.vector.tensor_scalar(out=pb[0:Mo], in0=qp[0:Mo],
                                        scalar1=1.0, scalar2=None, op0=OP.min)
                d = pool_big.tile([128, C, W], dt.bfloat16, tag="oh")
                nc.vector.tensor_tensor(out=d[0:Mo], in0=pb[0:Mo],
                                        in1=tb[0:Mo], op=OP.subtract)
                sq = pool_big.tile([128, C, W], dt.bfloat16, tag="qt")
                nc.vector.scalar_tensor_tensor(
                    out=sq[0:Mo], in0=d[0:Mo], scalar=1.0, in1=d[0:Mo],
                    op0=OP.mult, op1=OP.mult,
                    accum_out=acc[0:Mo, bi:bi + 1])

            tot = pool_cst.tile([128, 1], dt.float32, tag="tot")
            nc.vector.tensor_reduce(tot[:], acc[:], axis=mybir.AxisListType.X,
                                    op=OP.add)
            nc.sync.dma_start(out_ap[:], tot[:])

    nc.compile()
    _NC_CACHE = nc
    return nc


def kernel(pred: np.ndarray, target: np.ndarray) -> np.ndarray:
    assert pred.shape == (B, C, H, W) and target.shape == (B, H, W)
    nc = _build()
    in_maps = [
        {"pred": np.ascontiguousarray(pred[b]),
         "target": np.ascontiguousarray(target[b])}
        for b in range(N_CORES)
    ]
    res = run_bass_kernel_spmd(nc, in_maps, list(range(N_CORES)))
    total = sum(float(r["out"].sum()) for r in res.results)
    return np.float32(total / (B * C * H * W))


# revision 6
# speedup vs baseline: 3.0961x; 1.0338x over previous
"""BoundaryLoss Trainium2 kernel.

Computes mean((B(softmax(pred)) - B(onehot(target)))^2) where B is
clip(|3x3-Laplacian|, 0, 1) per (batch, class) plane.

Data parallel over batch: one batch element per NeuronCore (8 cores).
Per core, rows-on-partitions layout; H=512 in 5 bands (126*4+8 output rows),
each band loads its input rows plus halo. The Laplacian 9x - S_h(S_w(x)) is
3 TensorE matmuls per plane: banded weights give S_h over partitions, rhs
free-dim offsets +-1 give S_w. ACT evacuates |y| from PSUM two planes at a
time; DVE does softmax (bf16 tree sum, Ln/Exp reciprocal on ACT), onehot
(tensor_scalar is_equal on bf16 labels), clip, diff and the d^2 accumulation.
Host sums the per-partition partials and divides.
"""

import os
import numpy as np
import ml_dtypes
from contextlib import ExitStack

import concourse.bass as bass
import concourse.tile as tile
from concourse import bacc, mybir
from concourse.bass_utils import run_bass_kernel_spmd

N_CORES = int(os.environ.get("K_CORES", "8"))
B, C, H, W = 8, 19, 512, 512
dt = mybir.dt
AF = mybir.ActivationFunctionType
OP = mybir.AluOpType

# band = (h_in_lo, P_in, M_out, shift)
BANDS = [
    (0, 128, 126, 0),
    (125, 128, 126, 1),
    (251, 128, 126, 1),
    (377, 128, 126, 1),
    (503, 9, 8, 1),
]

PAIRS = [(c, c + 1) for c in range(0, C - 1, 2)] + [(C - 1,)]
CHUNKS = [(0, 4), (4, 4), (8, 4), (12, 4), (16, 3)]  # pred DMA/exp chunks


def _band_weights(P_in, M_out, shift):
    A = np.zeros((P_in, M_out), dtype=np.float32)
    E = np.zeros((P_in, M_out), dtype=np.float32)
    for m in range(M_out):
        for k in range(P_in):
            if abs(k - (m + shift)) <= 1:
                A[k, m] = 1.0
        E[m + shift, m] = 1.0
    w0 = (9.0 * E - A).astype(ml_dtypes.bfloat16)
    w1 = (-A).astype(ml_dtypes.bfloat16)
    return w0, w1


_NC_CACHE = None


def _build():
    global _NC_CACHE
    if _NC_CACHE is not None:
        return _NC_CACHE

    nc = bacc.Bacc("TRN2", target_bir_lowering=False, debug=False,
                   num_devices=N_CORES)

    pred_ap = nc.dram_tensor("pred", [C, H, W], dt.float32,
                             kind="ExternalInput").ap()
    tgt_ap = nc.dram_tensor("target", [H, W], dt.int32,
                            kind="ExternalInput").ap()
    out_ap = nc.dram_tensor("out", [128, 1], dt.float32,
                            kind="ExternalOutput").ap()

    w_drams = {}
    for key, (P_in, M_out, shift) in {
        "first": (128, 126, 0),
        "mid": (128, 126, 1),
        "last": (9, 8, 1),
    }.items():
        w0, w1 = _band_weights(P_in, M_out, shift)
        w_drams[key] = (nc.inline_tensor(w0, name=f"w0_{key}"),
                        nc.inline_tensor(w1, name=f"w1_{key}"))

    pred_v = pred_ap.transpose([1, 0, 2])  # [H, C, W] view of DRAM

    with tile.TileContext(nc) as tc:
        with ExitStack() as ctx:
            pool_pred = ctx.enter_context(tc.tile_pool(name="pred", bufs=3))
            pool_tgt = ctx.enter_context(tc.tile_pool(name="tgt", bufs=2))
            pool_big = ctx.enter_context(tc.tile_pool(name="big", bufs=1))
            pool_q = ctx.enter_context(tc.tile_pool(name="q", bufs=2))
            pool_sm = ctx.enter_context(tc.tile_pool(name="sm", bufs=2))
            pool_cst = ctx.enter_context(tc.tile_pool(name="cst", bufs=1))
            pool_ps = ctx.enter_context(
                tc.tile_pool(name="ps", bufs=2, space="PSUM"))

            w_sb = {}
            for key, (w0d, w1d) in w_drams.items():
                kk, mm = w0d.shape
                w0t = pool_cst.tile([kk, mm], dt.bfloat16, tag=f"w0{key}")
                w1t = pool_cst.tile([kk, mm], dt.bfloat16, tag=f"w1{key}")
                nc.sync.dma_start(w0t[:], w0d.ap()[:])
                nc.sync.dma_start(w1t[:], w1d.ap()[:])
                w_sb[key] = (w0t, w1t)

            acc = pool_cst.tile([128, len(BANDS)], dt.float32, tag="acc")
            nc.vector.memset(acc[:], 0.0)

            for bi, (h_lo, P_in, M_out, shift) in enumerate(BANDS):
                key = "first" if bi == 0 else ("last" if P_in < 128 else "mid")
                w0t, w1t = w_sb[key]
                Pi, Mo = P_in, M_out

                # chunked pred DMA + exp (f32 -> bf16)
                e = pool_big.tile([128, C, W], dt.bfloat16, tag="e")
                for c0, nch in CHUNKS:
                    pch = pool_pred.tile([128, 4, W], dt.float32, tag="pred")
                    nc.sync.dma_start(
                        pch[0:Pi, 0:nch, :],
                        pred_v[h_lo:h_lo + Pi, c0:c0 + nch, :])
                    nc.scalar.activation(e[0:Pi, c0:c0 + nch, :],
                                         pch[0:Pi, 0:nch, :], AF.Exp)
                tgtt = pool_tgt.tile([128, W], dt.int32, tag="tgt")
                nc.sync.dma_start(tgtt[0:Pi], tgt_ap[h_lo:h_lo + Pi])

                # bf16 pairwise-tree sum of the 19 planes (scratch in q pool)
                ts = pool_q.tile([128, C, W], dt.bfloat16, tag="qt")
                nc.vector.tensor_tensor(out=ts[0:Pi, 0:9, :], in0=e[0:Pi, 0:9, :],
                                        in1=e[0:Pi, 9:18, :], op=OP.add)
                nc.vector.tensor_tensor(out=ts[0:Pi, 0:4, :], in0=ts[0:Pi, 0:4, :],
                                        in1=ts[0:Pi, 4:8, :], op=OP.add)
                nc.vector.tensor_tensor(out=ts[0:Pi, 0:2, :], in0=ts[0:Pi, 0:2, :],
                                        in1=ts[0:Pi, 2:4, :], op=OP.add)
                nc.vector.tensor_tensor(out=ts[0:Pi, 0, :], in0=ts[0:Pi, 0, :],
                                        in1=ts[0:Pi, 1, :], op=OP.add)
                nc.vector.tensor_tensor(out=ts[0:Pi, 0, :], in0=ts[0:Pi, 0, :],
                                        in1=ts[0:Pi, 8, :], op=OP.add)
                S = pool_sm.tile([128, W], dt.float32, tag="S")
                nc.vector.tensor_tensor(out=S[0:Pi], in0=ts[0:Pi, 0, :],
                                        in1=e[0:Pi, 18, :], op=OP.add)

                # R = 1/S via exp(-ln(S)) on ACT, straight to bf16
                lnS = pool_sm.tile([128, W], dt.float32, tag="lnS")
                nc.scalar.activation(lnS[0:Pi], S[0:Pi], AF.Ln)
                Rb = pool_sm.tile([128, W], dt.bfloat16, tag="Rb")
                nc.scalar.activation(Rb[0:Pi], lnS[0:Pi], AF.Exp, scale=-1.0)

                # p = e * R, per class (keeps DVE in 2x bf16 mode)
                p = pool_big.tile([128, C, W], dt.bfloat16, tag="p")
                for c in range(C):
                    nc.vector.tensor_tensor(out=p[0:Pi, c, :], in0=e[0:Pi, c, :],
                                            in1=Rb[0:Pi], op=OP.mult)

                # onehot: bf16 labels, TS is_equal per class (4x mode)
                tgtb = pool_sm.tile([128, W], dt.bfloat16, tag="tgtb")
                nc.vector.tensor_copy(tgtb[0:Pi], tgtt[0:Pi])
                oh = pool_big.tile([128, C, W], dt.bfloat16, tag="oh")
                for c in range(C):
                    nc.vector.tensor_scalar(out=oh[0:Pi, c, :], in0=tgtb[0:Pi],
                                            scalar1=float(c), scalar2=None,
                                            op0=OP.is_equal)

                # Laplacian on PE, two planes per PSUM group
                qp = pool_q.tile([128, C, W], dt.bfloat16, tag="qp")
                qt = pool_q.tile([128, C, W], dt.bfloat16, tag="qt")
                for pr in PAIRS:
                    pp = pool_ps.tile([126, 2, W], dt.float32, tag="pp")
                    pt = pool_ps.tile([126, 2, W], dt.float32, tag="pt")
                    for j, c in enumerate(pr):
                        nc.tensor.matmul(pp[0:Mo, j, :], lhsT=w0t[:],
                                         rhs=p[0:Pi, c, :],
                                         start=True, stop=False)
                        nc.tensor.matmul(pt[0:Mo, j, :], lhsT=w0t[:],
                                         rhs=oh[0:Pi, c, :],
                                         start=True, stop=False)
                    for src, ps in ((p, pp), (oh, pt)):
                        for j, c in enumerate(pr):
                            last = j == len(pr) - 1
                            nc.tensor.matmul(ps[0:Mo, j, 1:W], lhsT=w1t[:],
                                             rhs=src[0:Pi, c, 0:W - 1],
                                             start=False, stop=False)
                            nc.tensor.matmul(ps[0:Mo, j, 0:W - 1], lhsT=w1t[:],
                                             rhs=src[0:Pi, c, 1:W],
                                             start=False, stop=last)
                    n = len(pr)
                    c0 = pr[0]
                    nc.scalar.activation(qp[0:Mo, c0:c0 + n, :],
                                         pp[0:Mo, 0:n, :], AF.Abs)
                    nc.scalar.activation(qt[0:Mo, c0:c0 + n, :],
                                         pt[0:Mo, 0:n, :], AF.Abs)

                # tb = min(qt,1) in place; pb = min(qp,1) in place
                nc.vector.tensor_scalar(out=qt[0:Mo], in0=qt[0:Mo],
                                        scalar1=1.0, scalar2=None, op0=OP.min)
                nc.vector.tensor_scalar(out=qp[0:Mo], in0=qp[0:Mo],
                                        scalar1=1.0, scalar2=None, op0=OP.min)
                d = pool_big.tile([128, C, W], dt.bfloat16, tag="p")
                nc.vector.tensor_tensor(out=d[0:Mo], in0=qp[0:Mo],
                                        in1=qt[0:Mo], op=OP.subtract)
                sq = pool_big.tile([128, C, W], dt.bfloat16, tag="oh")
                if bi % 2 == 0:
                    nc.vector.scalar_tensor_tensor(
                        out=sq[0:Mo], in0=d[0:Mo], scalar=1.0, in1=d[0:Mo],
                        op0=OP.mult, op1=OP.mult,
                        accum_out=acc[0:Mo, bi:bi + 1])
                else:
                    nc.scalar.activation(sq[0:Mo], d[0:Mo], AF.Square,
                                         accum_out=acc[0:Mo, bi:bi + 1])

            tot = pool_cst.tile([128, 1], dt.float32, tag="tot")
            nc.vector.tensor_reduce(tot[:], acc[:], axis=mybir.AxisListType.X,
                                    op=OP.add)
            nc.sync.dma_start(out_ap[:], tot[:])

    nc.compile()
    _NC_CACHE = nc
    return nc


def kernel(pred: np.ndarray, target: np.ndarray) -> np.ndarray:
    assert pred.shape == (B, C, H, W) and target.shape == (B, H, W)
    nc = _build()
    in_maps = [
        {"pred": np.ascontiguousarray(pred[b]),
         "target": np.ascontiguousarray(target[b])}
        for b in range(N_CORES)
    ]
    res = run_bass_kernel_spmd(nc, in_maps, list(range(N_CORES)))
    total = sum(float(r["out"].sum()) for r in res.results)
    return np.float32(total / (B * C * H * W))


# revision 7
# speedup vs baseline: 3.2012x; 1.0339x over previous
"""BoundaryLoss Trainium2 kernel.

Computes mean((B(softmax(pred)) - B(onehot(target)))^2) where B is
clip(|3x3-Laplacian|, 0, 1) per (batch, class) plane.

Data parallel over batch: one batch element per NeuronCore (8 cores).
Per core, rows-on-partitions layout; H=512 in 5 bands (126*4+8 output rows),
each band loads its input rows plus halo. The Laplacian 9x - S_h(S_w(x)) is
3 TensorE matmuls per plane: banded weights give S_h over partitions, rhs
free-dim offsets +-1 give S_w. ACT evacuates |y| from PSUM two planes at a
time; DVE does softmax (bf16 tree sum, Ln/Exp reciprocal on ACT), onehot
(tensor_scalar is_equal on bf16 labels), clip, diff and the d^2 accumulation.
Host sums the per-partition partials and divides.
"""

import os
import numpy as np
import ml_dtypes
from contextlib import ExitStack

import concourse.bass as bass
import concourse.tile as tile
from concourse import bacc, mybir
from concourse.bass_utils import run_bass_kernel_spmd

N_CORES = int(os.environ.get("K_CORES", "8"))
B, C, H, W = 8, 19, 512, 512
dt = mybir.dt
AF = mybir.ActivationFunctionType
OP = mybir.AluOpType

# band = (h_in_lo, P_in, M_out, shift)
BANDS = [
    (0, 128, 126, 0),
    (125, 128, 126, 1),
    (251, 128, 126, 1),
    (377, 128, 126, 1),
    (503, 9, 8, 1),
]

PAIRS = [(c, c + 1) for c in range(0, C - 1, 2)] + [(C - 1,)]
CHUNKS = [(0, 4), (4, 4), (8, 4), (12, 4), (16, 3)]  # pred DMA/exp chunks


def _band_weights(P_in, M_out, shift):
    A = np.zeros((P_in, M_out), dtype=np.float32)
    E = np.zeros((P_in, M_out), dtype=np.float32)
    for m in range(M_out):
        for k in range(P_in):
            if abs(k - (m + shift)) <= 1:
                A[k, m] = 1.0
        E[m + shift, m] = 1.0
    w0 = (9.0 * E - A).astype(ml_dtypes.bfloat16)
    w1 = (-A).astype(ml_dtypes.bfloat16)
    return w0, w1


_NC_CACHE = None


def _build():
    global _NC_CACHE
    if _NC_CACHE is not None:
        return _NC_CACHE

    nc = bacc.Bacc("TRN2", target_bir_lowering=False, debug=False,
                   num_devices=N_CORES)

    pred_ap = nc.dram_tensor("pred", [C, H, W], dt.float32,
                             kind="ExternalInput").ap()
    tgt_ap = nc.dram_tensor("target", [H, W], dt.int32,
                            kind="ExternalInput").ap()
    out_ap = nc.dram_tensor("out", [128, 1], dt.float32,
                            kind="ExternalOutput").ap()

    w_drams = {}
    for key, (P_in, M_out, shift) in {
        "first": (128, 126, 0),
        "mid": (128, 126, 1),
        "last": (9, 8, 1),
    }.items():
        w0, w1 = _band_weights(P_in, M_out, shift)
        w_drams[key] = (nc.inline_tensor(w0, name=f"w0_{key}"),
                        nc.inline_tensor(w1, name=f"w1_{key}"))

    pred_v = pred_ap.transpose([1, 0, 2])  # [H, C, W] view of DRAM

    with tile.TileContext(nc) as tc:
        with ExitStack() as ctx:
            pool_pred = ctx.enter_context(tc.tile_pool(name="pred", bufs=2))
            pool_tgt = ctx.enter_context(tc.tile_pool(name="tgt", bufs=2))
            pool_big = ctx.enter_context(tc.tile_pool(name="big", bufs=1))
            pool_oh = ctx.enter_context(tc.tile_pool(name="ohp", bufs=2))
            pool_q = ctx.enter_context(tc.tile_pool(name="q", bufs=2))
            pool_sm = ctx.enter_context(tc.tile_pool(name="sm", bufs=1))
            pool_cst = ctx.enter_context(tc.tile_pool(name="cst", bufs=1))
            pool_ps = ctx.enter_context(
                tc.tile_pool(name="ps", bufs=2, space="PSUM"))

            w_sb = {}
            for key, (w0d, w1d) in w_drams.items():
                kk, mm = w0d.shape
                w0t = pool_cst.tile([kk, mm], dt.bfloat16, tag=f"w0{key}")
                w1t = pool_cst.tile([kk, mm], dt.bfloat16, tag=f"w1{key}")
                nc.sync.dma_start(w0t[:], w0d.ap()[:])
                nc.sync.dma_start(w1t[:], w1d.ap()[:])
                w_sb[key] = (w0t, w1t)

            acc = pool_cst.tile([128, len(BANDS)], dt.float32, tag="acc")
            nc.vector.memset(acc[:], 0.0)

            for bi, (h_lo, P_in, M_out, shift) in enumerate(BANDS):
                key = "first" if bi == 0 else ("last" if P_in < 128 else "mid")
                w0t, w1t = w_sb[key]
                Pi, Mo = P_in, M_out

                # ---- t path first: it is independent of the softmax chain ----
                tgtt = pool_tgt.tile([128, W], dt.int32, tag="tgt")
                nc.sync.dma_start(tgtt[0:Pi], tgt_ap[h_lo:h_lo + Pi])
                tgtb = pool_sm.tile([128, W], dt.bfloat16, tag="tgtb")
                nc.vector.tensor_copy(tgtb[0:Pi], tgtt[0:Pi])
                oh = pool_oh.tile([128, C, W], dt.bfloat16, tag="oh")
                for c in range(C):
                    nc.vector.tensor_scalar(out=oh[0:Pi, c, :], in0=tgtb[0:Pi],
                                            scalar1=float(c), scalar2=None,
                                            op0=OP.is_equal)

                qt = pool_q.tile([128, C, W], dt.bfloat16, tag="qt")
                for pr in PAIRS:
                    pt = pool_ps.tile([126, 2, W], dt.float32, tag="pt")
                    for j, c in enumerate(pr):
                        nc.tensor.matmul(pt[0:Mo, j, :], lhsT=w0t[:],
                                         rhs=oh[0:Pi, c, :],
                                         start=True, stop=False)
                    for j, c in enumerate(pr):
                        last = j == len(pr) - 1
                        nc.tensor.matmul(pt[0:Mo, j, 1:W], lhsT=w1t[:],
                                         rhs=oh[0:Pi, c, 0:W - 1],
                                         start=False, stop=False)
                        nc.tensor.matmul(pt[0:Mo, j, 0:W - 1], lhsT=w1t[:],
                                         rhs=oh[0:Pi, c, 1:W],
                                         start=False, stop=last)
                    n = len(pr)
                    c0 = pr[0]
                    nc.scalar.activation(qt[0:Mo, c0:c0 + n, :],
                                         pt[0:Mo, 0:n, :], AF.Abs)
                # tb = min(qt,1) in place
                nc.vector.tensor_scalar(out=qt[0:Mo], in0=qt[0:Mo],
                                        scalar1=1.0, scalar2=None, op0=OP.min)

                # ---- softmax path ----
                e = pool_big.tile([128, C, W], dt.bfloat16, tag="e")
                for c0, nch in CHUNKS:
                    pch = pool_pred.tile([128, 4, W], dt.float32, tag="pred")
                    nc.sync.dma_start(
                        pch[0:Pi, 0:nch, :],
                        pred_v[h_lo:h_lo + Pi, c0:c0 + nch, :])
                    nc.scalar.activation(e[0:Pi, c0:c0 + nch, :],
                                         pch[0:Pi, 0:nch, :], AF.Exp)

                # bf16 pairwise-tree sum of the 19 planes (scratch in oh tag)
                ts = pool_oh.tile([128, C, W], dt.bfloat16, tag="oh")
                nc.vector.tensor_tensor(out=ts[0:Pi, 0:9, :], in0=e[0:Pi, 0:9, :],
                                        in1=e[0:Pi, 9:18, :], op=OP.add)
                nc.vector.tensor_tensor(out=ts[0:Pi, 0:4, :], in0=ts[0:Pi, 0:4, :],
                                        in1=ts[0:Pi, 4:8, :], op=OP.add)
                nc.vector.tensor_tensor(out=ts[0:Pi, 0:2, :], in0=ts[0:Pi, 0:2, :],
                                        in1=ts[0:Pi, 2:4, :], op=OP.add)
                nc.vector.tensor_tensor(out=ts[0:Pi, 0, :], in0=ts[0:Pi, 0, :],
                                        in1=ts[0:Pi, 1, :], op=OP.add)
                nc.vector.tensor_tensor(out=ts[0:Pi, 0, :], in0=ts[0:Pi, 0, :],
                                        in1=ts[0:Pi, 8, :], op=OP.add)
                S = pool_sm.tile([128, W], dt.float32, tag="S")
                nc.vector.tensor_tensor(out=S[0:Pi], in0=ts[0:Pi, 0, :],
                                        in1=e[0:Pi, 18, :], op=OP.add)

                # R = 1/S via exp(-ln(S)) on ACT, straight to bf16
                lnS = pool_sm.tile([128, W], dt.float32, tag="lnS")
                nc.scalar.activation(lnS[0:Pi], S[0:Pi], AF.Ln)
                Rb = pool_sm.tile([128, W], dt.bfloat16, tag="Rb")
                nc.scalar.activation(Rb[0:Pi], lnS[0:Pi], AF.Exp, scale=-1.0)

                # p = e * R, per class (keeps DVE in 2x bf16 mode)
                p = pool_big.tile([128, C, W], dt.bfloat16, tag="p")
                for c in range(C):
                    nc.vector.tensor_tensor(out=p[0:Pi, c, :], in0=e[0:Pi, c, :],
                                            in1=Rb[0:Pi], op=OP.mult)

                qp = pool_q.tile([128, C, W], dt.bfloat16, tag="qp")
                for pr in PAIRS:
                    pp = pool_ps.tile([126, 2, W], dt.float32, tag="pp")
                    for j, c in enumerate(pr):
                        nc.tensor.matmul(pp[0:Mo, j, :], lhsT=w0t[:],
                                         rhs=p[0:Pi, c, :],
                                         start=True, stop=False)
                    for j, c in enumerate(pr):
                        last = j == len(pr) - 1
                        nc.tensor.matmul(pp[0:Mo, j, 1:W], lhsT=w1t[:],
                                         rhs=p[0:Pi, c, 0:W - 1],
                                         start=False, stop=False)
                        nc.tensor.matmul(pp[0:Mo, j, 0:W - 1], lhsT=w1t[:],
                                         rhs=p[0:Pi, c, 1:W],
                                         start=False, stop=last)
                    n = len(pr)
                    c0 = pr[0]
                    nc.scalar.activation(qp[0:Mo, c0:c0 + n, :],
                                         pp[0:Mo, 0:n, :], AF.Abs)

                # pb = min(qp,1) in place; d = pb - tb; acc += sum(d^2)
                nc.vector.tensor_scalar(out=qp[0:Mo], in0=qp[0:Mo],
                                        scalar1=1.0, scalar2=None, op0=OP.min)
                d = pool_big.tile([128, C, W], dt.bfloat16, tag="p")
                nc.vector.tensor_tensor(out=d[0:Mo], in0=qp[0:Mo],
                                        in1=qt[0:Mo], op=OP.subtract)
                sq = pool_oh.tile([128, C, W], dt.bfloat16, tag="oh")
                if bi % 2 == 0:
                    nc.vector.scalar_tensor_tensor(
                        out=sq[0:Mo], in0=d[0:Mo], scalar=1.0, in1=d[0:Mo],
                        op0=OP.mult, op1=OP.mult,
                        accum_out=acc[0:Mo, bi:bi + 1])
                else:
                    nc.scalar.activation(sq[0:Mo], d[0:Mo], AF.Square,
                                         accum_out=acc[0:Mo, bi:bi + 1])

            tot = pool_cst.tile([128, 1], dt.float32, tag="tot")
            nc.vector.tensor_reduce(tot[:], acc[:], axis=mybir.AxisListType.X,
                                    op=OP.add)
            nc.sync.dma_start(out_ap[:], tot[:])

    nc.compile()
    _NC_CACHE = nc
    return nc


def kernel(pred: np.ndarray, target: np.ndarray) -> np.ndarray:
    assert pred.shape == (B, C, H, W) and target.shape == (B, H, W)
    nc = _build()
    in_maps = [
        {"pred": np.ascontiguousarray(pred[b]),
         "target": np.ascontiguousarray(target[b])}
        for b in range(N_CORES)
    ]
    res = run_bass_kernel_spmd(nc, in_maps, list(range(N_CORES)))
    total = sum(float(r["out"].sum()) for r in res.results)
    return np.float32(total / (B * C * H * W))


# revision 8
# speedup vs baseline: 3.5112x; 1.0968x over previous
"""BoundaryLoss Trainium2 kernel.

Computes mean((B(softmax(pred)) - B(onehot(target)))^2) where B is
clip(|3x3-Laplacian|, 0, 1) per (batch, class) plane.

Data parallel over batch: one batch element per NeuronCore (8 cores).
Per core, rows-on-partitions layout; H=512 in 5 bands (126*4+8 output rows),
each band loads its input rows plus halo. The Laplacian 9x - S_h(S_w(x)) is
3 TensorE matmuls per plane: banded weights give S_h over partitions, rhs
free-dim offsets +-1 give S_w. ACT evacuates |y| from PSUM two planes at a
time; DVE does softmax (bf16 tree sum, Ln/Exp reciprocal on ACT), onehot
(tensor_scalar is_equal on bf16 labels), clip, diff and the d^2 accumulation.
Host sums the per-partition partials and divides.
"""

import os
import numpy as np
import ml_dtypes
from contextlib import ExitStack

import concourse.bass as bass
import concourse.tile as tile
from concourse import bacc, mybir
from concourse.bass_utils import run_bass_kernel_spmd

N_CORES = int(os.environ.get("K_CORES", "8"))
B, C, H, W = 8, 19, 512, 512
dt = mybir.dt
AF = mybir.ActivationFunctionType
OP = mybir.AluOpType

# band = (h_in_lo, P_in, M_out, shift)
BANDS = [
    (0, 128, 126, 0),
    (125, 128, 126, 1),
    (251, 128, 126, 1),
    (377, 128, 126, 1),
    (503, 9, 8, 1),
]

PAIRS = [(c, c + 1) for c in range(0, C - 1, 2)] + [(C - 1,)]
CHUNKS = [(0, 4), (4, 4), (8, 4), (12, 4), (16, 3)]  # pred DMA/exp chunks


def _band_weights(P_in, M_out, shift):
    A = np.zeros((P_in, M_out), dtype=np.float32)
    E = np.zeros((P_in, M_out), dtype=np.float32)
    for m in range(M_out):
        for k in range(P_in):
            if abs(k - (m + shift)) <= 1:
                A[k, m] = 1.0
        E[m + shift, m] = 1.0
    w0 = (9.0 * E - A).astype(ml_dtypes.bfloat16)
    w1 = (-A).astype(ml_dtypes.bfloat16)
    return w0, w1


_NC_CACHE = None


def _build():
    global _NC_CACHE
    if _NC_CACHE is not None:
        return _NC_CACHE

    nc = bacc.Bacc("TRN2", target_bir_lowering=False, debug=False,
                   num_devices=N_CORES)

    pred_ap = nc.dram_tensor("pred", [C, H, W], dt.float32,
                             kind="ExternalInput").ap()
    tgt_ap = nc.dram_tensor("target", [H, W], dt.int32,
                            kind="ExternalInput").ap()
    out_ap = nc.dram_tensor("out", [128, 1], dt.float32,
                            kind="ExternalOutput").ap()

    w_drams = {}
    for key, (P_in, M_out, shift) in {
        "first": (128, 126, 0),
        "mid": (128, 126, 1),
        "last": (9, 8, 1),
    }.items():
        w0, w1 = _band_weights(P_in, M_out, shift)
        w_drams[key] = (nc.inline_tensor(w0, name=f"w0_{key}"),
                        nc.inline_tensor(w1, name=f"w1_{key}"))

    pred_v = pred_ap.transpose([1, 0, 2])  # [H, C, W] view of DRAM

    with tile.TileContext(nc) as tc:
        with ExitStack() as ctx:
            pool_pred = ctx.enter_context(tc.tile_pool(name="pred", bufs=2))
            pool_tgt = ctx.enter_context(tc.tile_pool(name="tgt", bufs=2))
            pool_big = ctx.enter_context(tc.tile_pool(name="big", bufs=1))
            pool_oh = ctx.enter_context(tc.tile_pool(name="ohp", bufs=2))
            pool_q = ctx.enter_context(tc.tile_pool(name="q", bufs=2))
            pool_sm = ctx.enter_context(tc.tile_pool(name="sm", bufs=1))
            pool_cst = ctx.enter_context(tc.tile_pool(name="cst", bufs=1))
            pool_ps = ctx.enter_context(
                tc.tile_pool(name="ps", bufs=2, space="PSUM"))

            w_sb = {}
            for key, (w0d, w1d) in w_drams.items():
                kk, mm = w0d.shape
                w0t = pool_cst.tile([kk, mm], dt.bfloat16, tag=f"w0{key}")
                w1t = pool_cst.tile([kk, mm], dt.bfloat16, tag=f"w1{key}")
                nc.sync.dma_start(w0t[:], w0d.ap()[:])
                nc.sync.dma_start(w1t[:], w1d.ap()[:])
                w_sb[key] = (w0t, w1t)

            acc = pool_cst.tile([128, 64], dt.float32, tag="acc")
            nc.vector.memset(acc[:], 0.0)

            for bi, (h_lo, P_in, M_out, shift) in enumerate(BANDS):
                key = "first" if bi == 0 else ("last" if P_in < 128 else "mid")
                w0t, w1t = w_sb[key]
                Pi, Mo = P_in, M_out

                # ---- t path first: independent of the softmax chain ----
                tgtt = pool_tgt.tile([128, W], dt.int32, tag="tgt")
                nc.sync.dma_start(tgtt[0:Pi], tgt_ap[h_lo:h_lo + Pi])
                tgtb = pool_sm.tile([128, W], dt.bfloat16, tag="tgtb")
                nc.vector.tensor_copy(tgtb[0:Pi], tgtt[0:Pi])
                oh = pool_oh.tile([128, C, W], dt.bfloat16, tag="oh")
                for c in range(C):
                    nc.vector.tensor_scalar(out=oh[0:Pi, c, :], in0=tgtb[0:Pi],
                                            scalar1=float(c), scalar2=None,
                                            op0=OP.is_equal)

                qt = pool_q.tile([128, C, W], dt.bfloat16, tag="qt")
                for pr in PAIRS:
                    pt = pool_ps.tile([126, 2, W], dt.float32, tag="pt")
                    for j, c in enumerate(pr):
                        nc.tensor.matmul(pt[0:Mo, j, :], lhsT=w0t[:],
                                         rhs=oh[0:Pi, c, :],
                                         start=True, stop=False)
                    for j, c in enumerate(pr):
                        last = j == len(pr) - 1
                        nc.tensor.matmul(pt[0:Mo, j, 1:W], lhsT=w1t[:],
                                         rhs=oh[0:Pi, c, 0:W - 1],
                                         start=False, stop=False)
                        nc.tensor.matmul(pt[0:Mo, j, 0:W - 1], lhsT=w1t[:],
                                         rhs=oh[0:Pi, c, 1:W],
                                         start=False, stop=last)
                    n, c0 = len(pr), pr[0]
                    nc.scalar.activation(qt[0:Mo, c0:c0 + n, :],
                                         pt[0:Mo, 0:n, :], AF.Abs)
                    nc.vector.tensor_scalar(out=qt[0:Mo, c0:c0 + n, :],
                                            in0=qt[0:Mo, c0:c0 + n, :],
                                            scalar1=1.0, scalar2=None,
                                            op0=OP.min)

                # ---- softmax: chunked exp with rolling chunk sums ----
                e = pool_big.tile([128, C, W], dt.bfloat16, tag="e")
                csum = pool_sm.tile([128, 5, W], dt.bfloat16, tag="cs")
                sc = pool_sm.tile([128, 2, W], dt.bfloat16, tag="sc")
                for ci, (c0, nch) in enumerate(CHUNKS):
                    pch = pool_pred.tile([128, 4, W], dt.float32, tag="pred")
                    nc.sync.dma_start(
                        pch[0:Pi, 0:nch, :],
                        pred_v[h_lo:h_lo + Pi, c0:c0 + nch, :])
                    nc.scalar.activation(e[0:Pi, c0:c0 + nch, :],
                                         pch[0:Pi, 0:nch, :], AF.Exp)
                    if nch == 4:
                        nc.vector.tensor_tensor(out=sc[0:Pi],
                                                in0=e[0:Pi, c0:c0 + 2, :],
                                                in1=e[0:Pi, c0 + 2:c0 + 4, :],
                                                op=OP.add)
                        nc.vector.tensor_tensor(out=csum[0:Pi, ci, :],
                                                in0=sc[0:Pi, 0, :],
                                                in1=sc[0:Pi, 1, :], op=OP.add)
                    else:
                        nc.vector.tensor_tensor(out=sc[0:Pi, 0, :],
                                                in0=e[0:Pi, c0, :],
                                                in1=e[0:Pi, c0 + 1, :],
                                                op=OP.add)
                        nc.vector.tensor_tensor(out=csum[0:Pi, ci, :],
                                                in0=sc[0:Pi, 0, :],
                                                in1=e[0:Pi, c0 + 2, :],
                                                op=OP.add)
                nc.vector.tensor_tensor(out=sc[0:Pi, 0, :], in0=csum[0:Pi, 0, :],
                                        in1=csum[0:Pi, 1, :], op=OP.add)
                nc.vector.tensor_tensor(out=sc[0:Pi, 1, :], in0=csum[0:Pi, 2, :],
                                        in1=csum[0:Pi, 3, :], op=OP.add)
                nc.vector.tensor_tensor(out=sc[0:Pi, 0, :], in0=sc[0:Pi, 0, :],
                                        in1=sc[0:Pi, 1, :], op=OP.add)
                S = pool_sm.tile([128, W], dt.float32, tag="S")
                nc.vector.tensor_tensor(out=S[0:Pi], in0=sc[0:Pi, 0, :],
                                        in1=csum[0:Pi, 4, :], op=OP.add)

                # R = 1/S via exp(-ln(S)) on ACT, straight to bf16
                lnS = pool_sm.tile([128, W], dt.float32, tag="lnS")
                nc.scalar.activation(lnS[0:Pi], S[0:Pi], AF.Ln)
                Rb = pool_sm.tile([128, W], dt.bfloat16, tag="Rb")
                nc.scalar.activation(Rb[0:Pi], lnS[0:Pi], AF.Exp, scale=-1.0)

                # ---- p path, fully pipelined per class-pair ----
                p = pool_big.tile([128, C, W], dt.bfloat16, tag="p")
                qp = pool_q.tile([128, C, W], dt.bfloat16, tag="qp")
                sq = pool_oh.tile([128, C, W], dt.bfloat16, tag="oh")
                for pi_, pr in enumerate(PAIRS):
                    for c in pr:
                        nc.vector.tensor_tensor(out=p[0:Pi, c, :],
                                                in0=e[0:Pi, c, :],
                                                in1=Rb[0:Pi], op=OP.mult)
                    pp = pool_ps.tile([126, 2, W], dt.float32, tag="pp")
                    for j, c in enumerate(pr):
                        nc.tensor.matmul(pp[0:Mo, j, :], lhsT=w0t[:],
                                         rhs=p[0:Pi, c, :],
                                         start=True, stop=False)
                    for j, c in enumerate(pr):
                        last = j == len(pr) - 1
                        nc.tensor.matmul(pp[0:Mo, j, 1:W], lhsT=w1t[:],
                                         rhs=p[0:Pi, c, 0:W - 1],
                                         start=False, stop=False)
                        nc.tensor.matmul(pp[0:Mo, j, 0:W - 1], lhsT=w1t[:],
                                         rhs=p[0:Pi, c, 1:W],
                                         start=False, stop=last)
                    n, c0 = len(pr), pr[0]
                    nc.scalar.activation(qp[0:Mo, c0:c0 + n, :],
                                         pp[0:Mo, 0:n, :], AF.Abs)
                    nc.vector.tensor_scalar(out=qp[0:Mo, c0:c0 + n, :],
                                            in0=qp[0:Mo, c0:c0 + n, :],
                                            scalar1=1.0, scalar2=None,
                                            op0=OP.min)
                    nc.vector.tensor_tensor(out=p[0:Mo, c0:c0 + n, :],
                                            in0=qp[0:Mo, c0:c0 + n, :],
                                            in1=qt[0:Mo, c0:c0 + n, :],
                                            op=OP.subtract)
                    slot = bi * 10 + pi_
                    if bi % 2 == 0:
                        nc.vector.scalar_tensor_tensor(
                            out=sq[0:Mo, c0:c0 + n, :],
                            in0=p[0:Mo, c0:c0 + n, :], scalar=1.0,
                            in1=p[0:Mo, c0:c0 + n, :],
                            op0=OP.mult, op1=OP.mult,
                            accum_out=acc[0:Mo, slot:slot + 1])
                    else:
                        nc.scalar.activation(sq[0:Mo, c0:c0 + n, :],
                                             p[0:Mo, c0:c0 + n, :], AF.Square,
                                             accum_out=acc[0:Mo, slot:slot + 1])

            tot = pool_cst.tile([128, 1], dt.float32, tag="tot")
            nc.vector.tensor_reduce(tot[:], acc[:], axis=mybir.AxisListType.X,
                                    op=OP.add)
            nc.sync.dma_start(out_ap[:], tot[:])

    nc.compile()
    _NC_CACHE = nc
    return nc


def kernel(pred: np.ndarray, target: np.ndarray) -> np.ndarray:
    assert pred.shape == (B, C, H, W) and target.shape == (B, H, W)
    nc = _build()
    in_maps = [
        {"pred": np.ascontiguousarray(pred[b]),
         "target": np.ascontiguousarray(target[b])}
        for b in range(N_CORES)
    ]
    res = run_bass_kernel_spmd(nc, in_maps, list(range(N_CORES)))
    total = sum(float(r["out"].sum()) for r in res.results)
    return np.float32(total / (B * C * H * W))
